# revision 85
# baseline (speedup 1.0000x reference)
"""Trainium2 Bass kernel for nn_CamAttnCon (topk-masked CAM attention consolidation).

Computation (per sample b):
  w[t]   = cosine(target_embed[b,t,:], fore_rep_encoded[b,:])     (masked where tgt<=0)
  top-k  = indices of the m largest w (m = min(ceil(0.1*seqlen), 51))
  total  = mean over top-m of relu(w[t]) * mean_h(align_attns[2][b,:,t,:])
  out    = minmax-normalize(total)                                 [B, S]

Strategy: pure data-parallel over batch; 4 samples per core on 8 cores.
Final version: fp16 embeddings (host-cast, halving the load DMA), two
half-batches of 2 samples whose cosine/topk phases overlap (half 0's
rank/gather pipeline runs during half 1's matmuls), squares split
ACT/DVE to track DMA arrival, exact top-k by rank with f32 compares
(fp16/bf16/f32r would tie or round differently between the broadcast and
its transpose and corrupt the strict-compare ranks / one-hot compaction),
masking folded into the f32 broadcast matmul (lhsT rows [+1; -1] over a
combined [w; mask] rhs tile), fp16 one-hot compaction matmuls, per-sample
indirect gathers of only the selected attention rows, and per-sample
weighted matmuls for the head/topk reduction at PSUM bases 0/32.
Per-sample scale factors (1/m, 1/H, 1/yn) are positive constants that
cancel exactly in the final min-max normalization and are skipped.
"""

import os
import sys

sys.path.insert(0, "/opt/trn_rl_repo")

import numpy as np
from contextlib import ExitStack

import concourse.bass as bass
import concourse.bacc as bacc
import concourse.mybir as mybir
import concourse.tile as tile
from concourse.masks import make_identity
from concourse import bass_utils

f32 = mybir.dt.float32
bf16 = mybir.dt.bfloat16
fp16 = mybir.dt.float16
i32 = mybir.dt.int32
AX = mybir.AxisListType
OP = mybir.AluOpType
AF = mybir.ActivationFunctionType

B, T, D, H, S = 32, 512, 512, 8, 196
NCORES = 8
BL = B // NCORES            # 4 samples per core
HB = 2                      # samples per half-batch
TC = T // 128               # 4 chunks of 128
HS = H * S                  # 1568
KK = int(0.1 * T)           # 51
J = 52                      # padded top-k slot count (>= KK)
JO = 64                     # partition base for the odd sample (PE constraint)
JP = JO + J                 # paired gather tile height (rows J..JO are padding)
EPS_NORM = 1e-12
BIG = 1e30

# square-engine assignment per (b, dc) flat index
SQ_ENG = ["act", "act", "dve", "dve",
          "act", "dve", "dve", "dve",
          "act", "dve", "dve", "dve",
          "act", "act", "dve", "dve"]

LAST_EXEC_NS = None
LAST_RESULTS = None


def build_body(ctx, tc, emb, att, fore, tgt, tgtT, out):
    nc = tc.nc

    # ---------------- pools ----------------
    const = ctx.enter_context(tc.tile_pool(name="const", bufs=1))
    small = ctx.enter_context(tc.tile_pool(name="small", bufs=1))
    embp = ctx.enter_context(tc.tile_pool(name="embp", bufs=1))
    sqp = ctx.enter_context(tc.tile_pool(name="sqp", bufs=1))
    wbcp = ctx.enter_context(tc.tile_pool(name="wbcp", bufs=2))
    cmpp = ctx.enter_context(tc.tile_pool(name="cmpp", bufs=4))
    gatp = ctx.enter_context(tc.tile_pool(name="gatp", bufs=4))

    ps_nx = ctx.enter_context(tc.tile_pool(name="ps_nx", bufs=1, space="PSUM"))
    ps_bc = ctx.enter_context(tc.tile_pool(name="ps_bc", bufs=2, space="PSUM"))
    ps_sm = ctx.enter_context(tc.tile_pool(name="ps_sm", bufs=1, space="PSUM"))
    ps_tot = ctx.enter_context(tc.tile_pool(name="ps_tot", bufs=1, space="PSUM"))

    # ---------------- ACT table warmup first (keep ACT.SEQ clear) ----------
    warm = const.tile([1, 1], f32, tag="warm")
    nc.vector.memset(warm[:], 1.0)
    warm2 = const.tile([1, 1], f32, tag="warm2")
    nc.scalar.sqrt(warm2[:], warm[:])
    nc.scalar.activation(out=warm2[:], in_=warm[:], func=AF.Square)
    nc.scalar.copy(warm2[:], warm[:])

    # ---------------- input DMAs (fore first: first matmul needs it) ------
    fore_sel = small.tile([128, TC * BL * HB], fp16, tag="fore_sel")
    nc.sync.dma_start(fore_sel[:], fore[:])  # fp16 host-side
    tgtT_sb = small.tile([128, TC * BL], i32, tag="tgtT")
    nc.gpsimd.dma_start(tgtT_sb[:], tgtT[:])
    tgt_h = []
    for h in range(2):
        th = small.tile([HB, T], i32, tag=f"tgt{h}")
        nc.gpsimd.dma_start(th[:], tgt[HB * h : HB * (h + 1), :])
        tgt_h.append(th)
    embR = emb.rearrange("b (dc p) t -> b p dc t", p=128)
    embt = []
    for b in range(BL):
        e = embp.tile([128, TC * T], fp16, tag=f"emb{b}")
        ev = e[:].rearrange("p (dc t) -> p dc t", dc=TC)
        nc.sync.dma_start(ev[:, 0:1, :], embR[b][:, 0:1, :])
        nc.sync.dma_start(ev[:, 1:2, :], embR[b][:, 1:2, :])
        if b == 3:
            nc.sync.dma_start(ev[:, 2:3, :], embR[b][:, 2:3, :])
            nc.sync.dma_start(ev[:, 3:4, :], embR[b][:, 3:4, :])
        else:
            nc.sync.dma_start(ev[:, 2:4, :], embR[b][:, 2:4, :])
        embt.append(e)

    # ---------------- constants ----------------
    id128 = const.tile([128, 128], f32, tag="id128")
    make_identity(nc, id128[:])
    # onesel2[:, bb*HB+bb] = 1 (xn2 selector for sample-in-half bb)
    onesel2 = const.tile([128, HB * HB], fp16, tag="onesel2")
    nc.vector.memset(onesel2[:], 0.0)
    for bb in range(HB):
        nc.vector.memset(onesel2[:, bb * HB + bb : bb * HB + bb + 1], 1.0)
    onesM = const.tile([1, 128], f32, tag="onesM")
    nc.vector.memset(onesM[:], 1.0)
    id2h = const.tile([2, 2], fp16, tag="id2h")
    nc.vector.tensor_copy(id2h[:], id128[0:2, 0:2])
    # bcsel2: lhsT [HB,128] slice bb = row bb all-ones (partition bcast sel)
    bcsel2 = const.tile([HB, HB * 128], f32, tag="bcsel2")
    nc.gpsimd.memset(bcsel2[:], 0.0)
    nc.gpsimd.affine_select(
        out=bcsel2[:].rearrange("p (blk j) -> p blk j", blk=HB),
        in_=bcsel2[:].rearrange("p (blk j) -> p blk j", blk=HB),
        compare_op=OP.not_equal,
        fill=1.0,
        base=0,
        pattern=[[-1, HB], [0, 128]],
        channel_multiplier=1,
    )

    bcsel34 = const.tile([34, HB * 128], f32, tag="bcsel34")
    nc.vector.memset(bcsel34[:], 0.0)
    nc.vector.tensor_copy(bcsel34[0:2, :], bcsel2[:])
    nc.vector.tensor_scalar(
        out=bcsel34[32:34, :], in0=bcsel2[:], scalar1=-1.0, scalar2=None,
        op0=OP.mult,
    )

    tv_i = const.tile([128, TC], i32, tag="tv_i")
    nc.gpsimd.iota(tv_i[:], pattern=[[128, TC]], base=0, channel_multiplier=1)
    tv_h = const.tile([128, TC], fp16, tag="tv_h")
    nc.vector.tensor_copy(tv_h[:], tv_i[:])

    jv_i = const.tile([128, J], i32, tag="jv_i")
    nc.gpsimd.iota(jv_i[:], pattern=[[1, J]], base=0, channel_multiplier=0)
    jv_h = const.tile([128, J], fp16, tag="jv_h")
    nc.vector.tensor_copy(jv_h[:], jv_i[:])
    # sign-rank slots for h0: raw = 2*rank - (T-1) -> slot values 2j-511
    jv_s = const.tile([128, J], fp16, tag="jv_s")
    nc.vector.tensor_scalar(
        out=jv_s[:], in0=jv_i[:], scalar1=2.0, scalar2=-float(T - 1),
        op0=OP.mult, op1=OP.add,
    )

    ten_i = const.tile([HB, KK], i32, tag="ten_i")
    nc.gpsimd.iota(ten_i[:], pattern=[[10, KK]], base=0, channel_multiplier=0)
    ten_f = const.tile([HB, KK], f32, tag="ten_f")
    nc.vector.tensor_copy(ten_f[:], ten_i[:])

    # boff2[:, b] = (T*b, 0): add sample-b row offset to the t row only
    boff2_i = const.tile([2, BL], i32, tag="boff2_i")
    nc.gpsimd.iota(boff2_i[:], pattern=[[T, BL]], base=0, channel_multiplier=0)
    boff2 = const.tile([2, BL], f32, tag="boff2")
    nc.vector.tensor_copy(boff2[:], boff2_i[:])
    pm2_i = const.tile([2, 1], i32, tag="pm2_i")
    nc.gpsimd.iota(pm2_i[:], pattern=[[1, 1]], base=0, channel_multiplier=1)
    pm2 = const.tile([2, 1], f32, tag="pm2")
    nc.vector.tensor_copy(pm2[:], pm2_i[:])
    nc.vector.tensor_scalar(
        out=pm2[:], in0=pm2[:], scalar1=1.0, scalar2=None, op0=OP.is_lt
    )
    nc.vector.tensor_scalar(
        out=boff2[:], in0=boff2[:], scalar1=pm2[:], scalar2=None, op0=OP.mult
    )

    # v2_b: per chunk c slot of 2 cols: col 0 = t-values, col 1 = g (late)
    v2t = []
    for b in range(BL):
        v2b = const.tile([128, TC * 2], fp16, tag=f"v2_{b}")
        for c in range(TC):
            nc.vector.tensor_copy(v2b[:, c * 2 : c * 2 + 1], tv_h[:, c : c + 1])
        v2t.append(v2b)

    # ---------------- mask / seqlen / m (per half, from int tgt) -----------
    mask_h, mbc_h, w4_t, maskT_t = [], [], [], []
    for h in range(2):
        tgt_f = small.tile([HB, T], f32, tag=f"tgtf{h}")
        nc.scalar.copy(tgt_f[:], tgt_h[h][:])
        mb01 = small.tile([HB, T], f32, tag=f"mb01_{h}")
        seqneg = small.tile([HB, 1], f32, tag=f"seqneg{h}")
        # invalid indicator (tgt <= 0); fused accum counts invalids per row
        nc.vector.tensor_scalar(
            out=mb01[:], in0=tgt_f[:], scalar1=0.0, scalar2=None,
            op0=OP.is_le, op1=OP.add, accum_out=seqneg[:],
        )
        # w4: rows 0-1 = w (written later), rows 32-33 = mask bias
        w4 = small.tile([34, T], f32, tag=f"w4_{h}")
        nc.vector.memset(w4[:], 0.0)
        nc.vector.tensor_scalar(
            out=w4[32:34, :], in0=mb01[:], scalar1=BIG, scalar2=None,
            op0=OP.mult,
        )
        nc.vector.memset(w4[32:34, 0:1], 0.0)
        w4_t.append(w4)
        # maskT: column-layout mask bias from host-transposed tgt
        mT = small.tile([128, TC * HB], f32, tag=f"maskT{h}")
        nc.vector.tensor_scalar(
            out=mT[:],
            in0=tgtT_sb[:].rearrange("p (c b) -> p c b", b=BL)[
                :, :, HB * h : HB * h + HB
            ],
            scalar1=0.0, scalar2=BIG, op0=OP.is_le, op1=OP.mult,
        )
        nc.vector.memset(mT[0:1, 0:HB], 0.0)
        maskT_t.append(mT)
        # seqlen = T - n_invalid, then +1 back if position 0 was counted
        # invalid (reference forces the first position valid)
        seqcol = small.tile([HB, 1], f32, tag=f"seqcol{h}")
        nc.vector.tensor_scalar(
            out=seqcol[:], in0=seqneg[:], scalar1=-1.0, scalar2=float(T),
            op0=OP.mult, op1=OP.add,
        )
        c0 = small.tile([HB, 1], f32, tag=f"c0{h}")
        nc.vector.tensor_scalar(
            out=c0[:], in0=tgt_f[:, 0:1], scalar1=0.0, scalar2=None,
            op0=OP.is_le,
        )
        nc.vector.tensor_tensor(seqcol[:], seqcol[:], c0[:], op=OP.add)
        # m = min(ceil(0.1*seqlen), KK) = sum_i [10*i < seqlen], i in [0, KK)
        mcnt = small.tile([HB, KK], f32, tag=f"mcnt{h}")
        nc.vector.tensor_scalar(
            out=mcnt[:], in0=ten_f[:], scalar1=seqcol[:], scalar2=None,
            op0=OP.is_lt,
        )
        mcol = small.tile([HB, 1], f32, tag=f"mcol{h}")
        nc.vector.tensor_reduce(mcol[:], mcnt[:], axis=AX.X, op=OP.add)
        mr_ps = ps_sm.tile([1, HB], f32, tag="tsm")
        nc.tensor.transpose(mr_ps[:], mcol[:], id128[0:HB, 0:HB])
        mrow = small.tile([1, HB], f32, tag=f"mrow{h}")
        nc.vector.tensor_copy(mrow[:], mr_ps[:])
        mbc_ps = ps_sm.tile([128, HB], f32, tag="tsm")
        nc.tensor.matmul(
            out=mbc_ps[:], lhsT=onesM[:], rhs=mrow[:], start=True, stop=True
        )
        mbc = small.tile([128, HB], f32, tag=f"mbc{h}")
        nc.vector.tensor_copy(mbc[:], mbc_ps[:])
        mask_h.append(None)
        mbc_h.append(mbc)

    # ---------------- half-batch pipeline ----------------

    nxh_t = []
    for h in range(2):
        # --- squares + cosine matmuls: num rows 0-1, xn2 rows 32-33 ---
        nxh = ps_nx.tile([34, T], f32, tag=f"nx{h}")
        nxh_t.append(nxh)
        for bb in range(HB):
            b = HB * h + bb
            sq = sqp.tile([128, TC * T], fp16, tag=f"sq{b}")
            for dc in range(TC):
                x = embt[b][:, dc * T : (dc + 1) * T]
                s = sq[:, dc * T : (dc + 1) * T]
                if SQ_ENG[b * TC + dc] == "act":
                    nc.scalar.activation(out=s, in_=x, func=AF.Square)
                else:
                    nc.vector.tensor_tensor(s, x, x, op=OP.mult)
                fs = (h * TC + dc) * HB + bb
                nc.tensor.matmul(
                    out=nxh[0:HB, :],
                    lhsT=fore_sel[:, fs * HB : (fs + 1) * HB],
                    rhs=x,
                    start=(bb == 0 and dc == 0),
                    stop=(bb == HB - 1 and dc == TC - 1),
                )
                nc.tensor.matmul(
                    out=nxh[32 : 32 + HB, :],
                    lhsT=onesel2[:, bb * HB : (bb + 1) * HB],
                    rhs=s,
                    start=(bb == 0 and dc == 0),
                    stop=(bb == HB - 1 and dc == TC - 1),
                )

    w_rows_t, wT_t, wTr_t = [], [], []
    for h in range(2):
        nxh = nxh_t[h]
        xn_rows = small.tile([HB, T], f32, tag=f"xn_rows{h}")
        nc.scalar.sqrt(xn_rows[:], nxh[32 : 32 + HB, :])
        rxn_rows = small.tile([HB, T], f32, tag=f"rxn_rows{h}")
        nc.vector.reciprocal(rxn_rows[:], xn_rows[:])
        w4 = w4_t[h]
        nc.vector.tensor_tensor(w4[0:HB, :], nxh[0:HB, :], rxn_rows[:], op=OP.mult)
        w_rows_t.append(w4)

        # wT columns [128, (c bb)] via 4 transposes (unmasked), then masked
        wT_ps = ps_sm.tile([128, TC * HB], f32, tag="tsm")
        for c in range(TC):
            nc.tensor.transpose(
                wT_ps[:, c * HB : (c + 1) * HB],
                w4[0:HB, c * 128 : (c + 1) * 128],
                id128[0:HB, 0:HB],
            )
        wT = small.tile([128, TC * HB], f32, tag=f"wT{h}")
        nc.vector.tensor_tensor(wT[:], wT_ps[:], maskT_t[h][:], op=OP.subtract)
        wT_t.append(wT)
        wTr = small.tile([128, TC * HB], fp16, tag=f"wTr{h}")
        nc.vector.tensor_scalar(
            out=wTr[:], in0=wT[:], scalar1=0.0, scalar2=None, op0=OP.max
        )
        wTr_t.append(wTr)

    tot_t = []
    for h in range(2):
        w_rows, wT, wTr = w_rows_t[h], wT_t[h], wTr_t[h]
        # sample bb of this half accumulates tot at partition base 32*bb
        tot_ps = ps_tot.tile([34, S], f32, tag=f"tot{h}")
        nc.vector.memset(tot_ps[:], 0.0)
        tot_t.append(tot_ps)
        # --- per-sample topk pipeline ---
        pairP = None
        for bb in range(HB):
            b = HB * h + bb
            wT_b = wT[:].rearrange("p (c bb) -> p c bb", bb=HB)[:, :, bb]

            wbc_ps = ps_bc.tile([128, T], f32, tag="bc")
            nc.tensor.matmul(
                out=wbc_ps[:],
                lhsT=bcsel34[:, bb * 128 : (bb + 1) * 128],
                rhs=w_rows[:],
                start=True,
                stop=True,
            )
            wbc_sb = wbcp.tile([128, T], f32, tag="wbc")
            nc.scalar.copy(wbc_sb[:], wbc_ps[:])
            # rank[q,c] = #{t' : w[t'] > w[c*128+q]} (compare+accum)
            rankT_b = small.tile([128, TC], f32, tag=f"rankT{b}")
            for c in range(TC):
                cmp_bf = cmpp.tile([128, T], bf16, tag="cmp")
                nc.vector.tensor_scalar(
                    out=cmp_bf[:],
                    in0=wbc_sb[:],
                    scalar1=wT_b[:, c : c + 1],
                    scalar2=None,
                    op0=OP.is_gt,
                    op1=OP.add,
                    accum_out=rankT_b[:, c : c + 1],
                )
            wTr_b = wTr[:].rearrange("p (c bb) -> p c bb", bb=HB)[:, :, bb]
            v2b = v2t[b]
            nc.vector.scalar_tensor_tensor(
                out=v2b[:].rearrange("p (c two) -> p c two", two=2)[:, :, 1],
                in0=rankT_b[:],
                scalar=mbc_h[h][:, bb : bb + 1],
                in1=wTr_b[:],
                op0=OP.is_lt,
                op1=OP.mult,
            )

            # one-hot compaction: stak2 rows = (compact t, compact g)
            st4 = cmpp.tile([128, TC * J], fp16, tag="st")
            nc.vector.tensor_tensor(
                out=st4[:].rearrange("p (c j) -> p c j", c=TC),
                in0=jv_h[:].unsqueeze(1).broadcast_to([128, TC, J]),
                in1=rankT_b[:].unsqueeze(2).broadcast_to([128, TC, J]),
                op=OP.is_equal,
            )
            stak2 = ps_sm.tile([2, J], f32, tag="tsm")
            for c in range(TC):
                nc.tensor.matmul(
                    out=stak2[:],
                    lhsT=v2b[:, c * 2 : (c + 1) * 2],
                    rhs=st4[:, c * J : (c + 1) * J],
                    start=(c == 0),
                    stop=(c == TC - 1),
                )
            stack2 = small.tile([2, J], f32, tag=f"stack2_{b}")
            if h == 0:
                nc.scalar.activation(
                    out=stack2[:],
                    in_=stak2[:],
                    func=AF.Relu,
                    bias=boff2[:, b : b + 1],
                    scale=1.0,
                )
            else:
                nc.vector.tensor_scalar(
                    out=stack2[:],
                    in0=stak2[:],
                    scalar1=boff2[:, b : b + 1],
                    scalar2=None,
                    op0=OP.add,
                )

            # transpose (t,g) rows into columns for this sample
            pstP = ps_sm.tile([J, 2], f32, tag="tsm")
            nc.tensor.transpose(pstP[:], stack2[:], id128[0:2, 0:2])

            # gather + weighted head/topk reduction for this sample
            idx_b = small.tile([J, 1], i32, tag=f"idxB{b}")
            gcol_b = small.tile([J, 1], fp16, tag=f"gcolB{b}")
            nc.scalar.copy(idx_b[:], pstP[:, 0:1])
            nc.scalar.copy(gcol_b[:], pstP[:, 1:2])
            gat_b = gatp.tile([J, HS], fp16, tag="gat")
            nc.gpsimd.indirect_dma_start(
                out=gat_b[:],
                out_offset=None,
                in_=att[:],
                in_offset=bass.IndirectOffsetOnAxis(ap=idx_b[:, 0:1], axis=0),
            )
            for hh in range(H):
                nc.tensor.matmul(
                    out=tot_ps[32 * bb : 32 * bb + 1, :],
                    lhsT=gcol_b[:],
                    rhs=gat_b[:, hh * S : (hh + 1) * S],
                    start=(hh == 0),
                    stop=(hh == H - 1),
                )

    # ---------------- normalize + store (final phase; overlaps by deps) ----
    for h in range(2):
        tp = tot_t[h][:]
        mn = small.tile([34, 1], f32, tag=f"mn{h}")
        nc.vector.tensor_reduce(mn[:], tp, axis=AX.X, op=OP.min)
        mx = small.tile([34, 1], f32, tag=f"mx{h}")
        nc.vector.tensor_reduce(mx[:], tp, axis=AX.X, op=OP.max)
        nc.vector.tensor_tensor(mx[:], mx[:], mn[:], op=OP.subtract)
        nc.vector.tensor_scalar_max(mx[:], mx[:], EPS_NORM)
        rmx = small.tile([34, 1], f32, tag=f"rmx{h}")
        nc.vector.reciprocal(rmx[:], mx[:])
        out_sb = small.tile([64, S], f32, tag=f"out_sb{h}")
        nc.vector.tensor_scalar(
            out=out_sb[0:34, :],
            in0=tp,
            scalar1=mn[:],
            scalar2=rmx[:],
            op0=OP.subtract,
            op1=OP.mult,
        )
        nc.sync.dma_start(
            out[2 * h : 2 * h + 2, :],
            out_sb[:].rearrange("(a c) s -> a c s", c=32)[:, 0, :],
        )


def build_nc(path=None):
    nc = bacc.Bacc("TRN2", target_bir_lowering=False, debug=False)
    emb = nc.dram_tensor("emb", [BL, D, T], fp16, kind="ExternalInput")
    att = nc.dram_tensor("att", [BL * T, HS], f32, kind="ExternalInput")
    fore = nc.dram_tensor("fore", [128, TC * BL * HB], fp16, kind="ExternalInput")
    tgt = nc.dram_tensor("tgt", [BL, T], i32, kind="ExternalInput")
    tgtT = nc.dram_tensor("tgtT", [128, TC * BL], i32, kind="ExternalInput")
    out = nc.dram_tensor("out", [BL, S], f32, kind="ExternalOutput")
    with ExitStack() as ctx:
        tc = ctx.enter_context(tile.TileContext(nc))
        build_body(ctx, tc, emb.ap(), att.ap(), fore.ap(), tgt.ap(), tgtT.ap(), out.ap())
    nc.compile()
    return nc


_NC_CACHE = {}


def get_nc(path=None):
    if "nc" not in _NC_CACHE:
        _NC_CACHE["nc"] = build_nc()
    return _NC_CACHE["nc"]


def make_in_maps(fore_rep_encoded, target_embed, align_attns, targets):
    LAYER_ID = 2
    att_l = np.transpose(np.asarray(align_attns[LAYER_ID]), (0, 2, 1, 3))  # [B,T,H,S]
    fore_all = np.asarray(fore_rep_encoded, dtype=np.float32)
    in_maps = []
    for cidx in range(NCORES):
        sl = slice(cidx * BL, (cidx + 1) * BL)
        # fore_sel block (h*TC+dc)*HB+bb: [128, HB] col bb = fore[2h+bb] chunk dc
        fsl = fore_all[sl]
        fc = np.zeros((128, TC * BL * HB), np.float16)
        for h in range(2):
            for dc in range(TC):
                for bb in range(HB):
                    blk = (h * TC + dc) * HB + bb
                    fc[:, blk * HB + bb] = fsl[HB * h + bb, dc * 128 : (dc + 1) * 128]
        in_maps.append(
            {
                "emb": np.ascontiguousarray(
                    np.swapaxes(np.asarray(target_embed)[sl], 1, 2), dtype=np.float16
                ),
                "att": np.ascontiguousarray(att_l[sl], dtype=np.float32).reshape(
                    BL * T, HS
                ),
                "fore": fc,
                "tgt": np.ascontiguousarray(np.asarray(targets)[sl, :T]).astype(
                    np.int32
                ),
                "tgtT": np.ascontiguousarray(
                    np.asarray(targets)[sl, :T]
                    .astype(np.int32)
                    .reshape(BL, TC, 128)
                    .transpose(2, 1, 0)
                    .reshape(128, TC * BL)
                ),
            }
        )
    return in_maps


def kernel(fore_rep_encoded, target_embed, align_attns, targets):
    global LAST_EXEC_NS, LAST_RESULTS
    nc = get_nc()
    in_maps = make_in_maps(fore_rep_encoded, target_embed, align_attns, targets)
    trace = bool(os.environ.get("KERNEL_TRACE"))
    try:
        res = bass_utils.run_bass_kernel_spmd(
            nc, in_maps, core_ids=list(range(NCORES)), trace=trace
        )
    except ModuleNotFoundError:
        # NTFF trace hook unavailable in this environment; run without trace
        os.environ["BASS_NEVER_TRACE"] = "1"
        res = bass_utils.run_bass_kernel_spmd(
            nc, in_maps, core_ids=list(range(NCORES)), trace=False
        )
    LAST_EXEC_NS = res.exec_time_ns
    LAST_RESULTS = res
    return np.concatenate([r["out"] for r in res.results], axis=0)
